# revision 29
# baseline (speedup 1.0000x reference)
"""Trainium2 Bass kernel for nn_Attention (B=2, N=2048, DIM=2048, H=16, HD=128).

Sharding: 8 cores = 2 batches x 4 head-groups (4 heads each). Each core:
  - QKV projection (fp16, token-partition layout); x / Wqkv / coef are
    host-pretiled so every load is one contiguous-per-partition DMA, with the
    tile-0 critical path quartered across the three DMA-capable queues
  - per-head RMSNorm + RoPE (channels host-deinterleaved to [even|odd] halves
    per head so every rope operand is a contiguous 16-bit run -> DVE 2x mode;
    the shared q/k permutation cancels in the scores), PE-transpose to [hd, n]
  - S^T = K^T.T @ Q^T scores, two key-tiles paired per PSUM tile so each ACT
    exp instruction runs 1024 elements per lane; es in bf16 (EXP_OFF keeps
    1/sums in fp16/bf16 range)
  - softmax sums via a DVE running sum over the es tiles (no per-tile PE
    ones-matmuls), one ones-matmul on the root, reciprocal_approx_fast,
    K=1 broadcast matmul; the normalize chain is split in two stages emitted
    a few pipeline steps apart so the PE never waits on the DVE
  - output projection emitted as filler groups interleaved into the NEXT
    chunk's attention (keeps PE busy while ACT works through the exps);
    fp16 partials, one consolidated store per token tile
Host sums the 4 head-group partials per batch.
"""

import sys

import numpy as np

sys.path.insert(0, "/opt/trn_rl_repo")

import ml_dtypes  # noqa: E402

import concourse.bass as bass  # noqa: E402
import concourse.tile as tile  # noqa: E402
from concourse import bacc  # noqa: E402
from concourse import mybir  # noqa: E402
from concourse.masks import make_identity  # noqa: E402

B, N, DIM, H, HD = 2, 2048, 2048, 16, 128
NCORES = 8
GROUPS = NCORES // B  # 4 head-groups
HPC = H // GROUPS  # 4 heads per core
CPC = HPC * HD  # 512 channels per core
EPS = 1e-5
SCALE = 1.0 / float(np.sqrt(HD))
EXP_OFF = -10.0  # keeps softmax sums in a reciprocal-friendly range; cancels

NT = N // 128  # 16 token tiles
DT = DIM // 128  # 16 contraction tiles
NJ = N // 512  # 4 query chunks
MP = NT // 2  # 8 key-tile pairs

F32 = mybir.dt.float32
F16 = mybir.dt.float16
BF16 = mybir.dt.bfloat16
AF = mybir.ActivationFunctionType


def _emit(tc: "tile.TileContext"):
    nc = tc.nc
    xT = nc.dram_tensor("xT", [128, 8, DT, 256], F16, kind="ExternalInput")
    wqkvT = nc.dram_tensor("wqkvT", [128, 3, DT, CPC], F16, kind="ExternalInput")
    woutT = nc.dram_tensor("woutT", [CPC, DIM], F16, kind="ExternalInput")
    coef = nc.dram_tensor("coef", [128, NT, 8, 2 * HD], F16, kind="ExternalInput")
    outp = nc.dram_tensor("outp", [N, DIM], F16, kind="ExternalOutput")

    with (
        tc.tile_pool(name="const", bufs=1) as const,
        tc.tile_pool(name="persist", bufs=1) as persist,
    ):
        ident = const.tile([128, 128], F16)
        make_identity(nc, ident)
        ones_col = const.tile([128, 1], BF16)
        nc.vector.memset(ones_col, 1.0)
        ones_row = const.tile([1, 128], BF16)
        nc.vector.memset(ones_row, 1.0)
        eps_sb = const.tile([128, 1], F32)
        nc.vector.memset(eps_sb, EPS)
        expoff_sb = const.tile([128, 1], F32)
        nc.vector.memset(expoff_sb, EXP_OFF)

        # persistent activations: QT/KT as [hd, head, 1024] halves, V token-major
        QT = [persist.tile([128, HPC, 1024], F16, tag=f"QT{c}", name=f"QT{c}") for c in range(2)]
        KT = [persist.tile([128, HPC, 1024], F16, tag=f"KT{c}", name=f"KT{c}") for c in range(2)]
        V = [persist.tile([128, 4, CPC], BF16, tag=f"V{c}", name=f"V{c}") for c in range(4)]
        O = [persist.tile([128, HPC, 512], F16, tag=f"O{j}", name=f"O{j}") for j in range(NJ)]
        wout_sb = persist.tile([128, HPC, DIM], F16, tag="wout")

        # ---------------- phase 1: QKV + rmsnorm + rope + transpose ------
        with (
            tc.tile_pool(name="wq", bufs=1) as wqp,
            tc.tile_pool(name="xs", bufs=3) as xsp,
            tc.tile_pool(name="cf", bufs=4) as cfp,
            tc.tile_pool(name="qn", bufs=2) as qnp,
            tc.tile_pool(name="qr", bufs=4) as qrp,
            tc.tile_pool(name="scr", bufs=2) as scrp,
            tc.tile_pool(name="qkv_ps", bufs=2, space="PSUM") as qkvps,
            tc.tile_pool(name="tr_ps", bufs=2, space="PSUM") as trps,
        ):
            # x / wqkv / coef are host-pretiled so every DMA is a single
            # contiguous-per-partition transfer at full bandwidth
            wq_big = wqp.tile([128, 3, DT, CPC], F16, tag="wq")
            wq_sb = [
                [wq_big[:, c, t, :] for t in range(DT)] for c in range(3)
            ]
            xref = {}
            dummy_exp = const.tile([1, 1], F32)

            def load_round(r, eng=None):
                xb = xsp.tile([128, DT, 256], F16, tag="xr", name="xb")
                (eng or nc.sync).dma_start(out=xb, in_=xT[:, r, :, :])
                xref[r] = [xb[:, t, :] for t in range(DT)]

            xb0 = xsp.tile([128, DT, 256], F16, tag="xr", name="xb0")
            q = [nc.sync, nc.scalar, nc.gpsimd]
            # tile-0 critical path: x round 0 and the first weight chunk,
            # quartered round-robin over all three DMA-capable queues
            xq = [nc.sync, nc.scalar, nc.gpsimd, nc.scalar]
            wq0q = [nc.sync, nc.scalar, nc.gpsimd, nc.gpsimd]
            for k in range(4):
                ds = slice(k * DT // 4, (k + 1) * DT // 4)
                xq[k].dma_start(out=xb0[:, ds], in_=xT[:, 0, ds])
            xref[0] = [xb0[:, t, :] for t in range(DT)]
            for k in range(4):
                ds = slice(k * DT // 4, (k + 1) * DT // 4)
                wq0q[k].dma_start(out=wq_big[:, 0, ds], in_=wqkvT[:, 0, ds])
            # coef tile 0 beats the remaining weight chunks; 1 and 2 queue
            # behind them (needed only from ~tile 1 onwards)
            cfb = {}

            def load_cf(i, eng):
                cf = cfp.tile([128, 8, 2 * HD], F16, tag="cf")
                eng.dma_start(out=cf, in_=coef[:, i, :, :])
                cfb[i] = cf

            load_cf(0, nc.gpsimd)
            for c in range(1, 3):
                for k in range(4):
                    ds = slice(k * DT // 4, (k + 1) * DT // 4)
                    q[(c + k) % 3].dma_start(
                        out=wq_big[:, c, ds], in_=wqkvT[:, c, ds]
                    )
            load_cf(1, nc.sync)
            load_cf(2, nc.gpsimd)
            load_round(1)
            load_round(2, nc.scalar)


            def transposes(i):
                qr = qr_tiles[i % 3]
                for qk in range(2):
                    trp = trps.tile([128, CPC], F16, tag="trp")
                    for hh in range(HPC):
                        hsl = slice(hh * HD, (hh + 1) * HD)
                        nc.tensor.transpose(trp[:, hsl], qr[:, qk, hsl], ident)
                    tgt = (QT if qk == 0 else KT)[i // 8]
                    dst = tgt[:, :, (i % 8) * 128 : (i % 8 + 1) * 128]
                    if i >= NT - 1:
                        nc.scalar.copy(
                            out=dst, in_=trp.rearrange("p (h n) -> p h n", h=HPC)
                        )
                    else:
                        nc.vector.tensor_copy(
                            out=dst, in_=trp.rearrange("p (h n) -> p h n", h=HPC)
                        )

            qr_tiles = {}
            for i in range(NT):
                nsl = slice(i * 128, (i + 1) * 128)
                ps = qkvps.tile([128, 3, CPC], F32, tag="qkv")
                # prefetch x three rounds ahead (tiles i+6, i+7)
                if i % 2 == 0 and i + 6 < NT:
                    load_round(i // 2 + 3)
                if i == 8:
                    # wout is first needed by outproj of chunk 0 (phase 2)
                    nc.sync.dma_start(
                        out=wout_sb,
                        in_=woutT.rearrange("(h p) d -> p h d", p=128),
                    )
                # coef prefetch 3 tiles ahead
                if i + 3 < NT:
                    cf3 = cfp.tile([128, 8, 2 * HD], F16, tag="cf")
                    nc.gpsimd.dma_start(out=cf3, in_=coef[:, i + 3, :, :])
                    cfb[i + 3] = cf3

                xsl = slice((i % 2) * 128, (i % 2 + 1) * 128)
                if i < 2:
                    # c-outer: first tiles start as soon as the q-chunk lands
                    for c in range(3):
                        for d in range(DT):
                            nc.tensor.matmul(
                                ps[:, c, :],
                                lhsT=xref[i // 2][d][:, xsl],
                                rhs=wq_sb[c][d],
                                start=(d == 0),
                                stop=(d == DT - 1),
                            )
                else:
                    for d in range(DT):
                        for c in range(3):
                            nc.tensor.matmul(
                                ps[:, c, :],
                                lhsT=xref[i // 2][d][:, xsl],
                                rhs=wq_sb[c][d],
                                start=(d == 0),
                                stop=(d == DT - 1),
                            )

                # V straight to SBUF (bf16)
                nc.vector.tensor_copy(out=V[i // 4][:, i % 4, :], in_=ps[:, 2, :])

                # rmsnorm: sum of squares per head -> rstd.  The last two
                # tiles compute ssq on DVE so the ACT queue is free for the
                # first attention exps right at the phase boundary.
                ssq = scrp.tile([128, 8], F32, tag="ssq")
                sq = scrp.tile([128, CPC], F16, tag="sq")
                for qk in range(2):
                    for hh in range(HPC):
                        hsl = slice(hh * HD, (hh + 1) * HD)
                        acc = ssq[:, qk * HPC + hh : qk * HPC + hh + 1]
                        nc.scalar.activation(
                            out=sq[:, hsl],
                            in_=ps[:, qk, hsl],
                            func=AF.Square,
                            accum_out=acc,
                        )
                rstd = scrp.tile([128, 8], F32, tag="rstd")
                nc.scalar.activation(
                    rstd, ssq, AF.Sqrt, bias=eps_sb, scale=1.0 / HD
                )
                nc.vector.reciprocal(rstd, rstd)
                if i == NT - 1:
                    # preload the Exp spline table while phase 1 drains
                    nc.scalar.activation(dummy_exp, eps_sb[0:1, :], AF.Exp)

                qn = qnp.tile([128, 2, CPC], F16, tag="qn")
                for qk in range(2):
                    for hh in range(HPC):
                        hsl = slice(hh * HD, (hh + 1) * HD)
                        nc.vector.tensor_scalar_mul(
                            out=qn[:, qk, hsl],
                            in0=ps[:, qk, hsl],
                            scalar1=rstd[:, qk * HPC + hh : qk * HPC + hh + 1],
                        )

                # transposes lag 2 tiles (after qn so the psum readers
                # lead the DVE queue and release qkv psum early)
                if i > 1:
                    transposes(i - 2)

                # rope (gammas folded into coefficients host-side)
                cf = cfb.pop(i)
                qr = qrp.tile([128, 2, CPC], F16, tag="qr")
                qr_tiles[i % 3] = qr
                # channels are host-permuted to [even(64) | odd(64)] per
                # head, so every rope operand is a contiguous run of 64 and
                # the DVE runs in its packed 16-bit 2x mode
                for qk in range(2):
                    base = qk * 4
                    half = qn[:, qk, :].rearrange("p (h t c) -> p h t c", t=2, c=64)
                    x0 = half[:, :, 0, :]
                    x1 = half[:, :, 1, :]
                    rot = qr[:, qk, :].rearrange("p (h t c) -> p h t c", t=2, c=64)
                    cfv = [
                        cf[:, base + r, :].rearrange("p (h c) -> p h c", c=64)
                        for r in range(4)
                    ]
                    ta = scrp.tile([128, HPC, 64], F16, tag="ta")
                    tb = scrp.tile([128, HPC, 64], F16, tag="tb")
                    nc.vector.tensor_mul(ta, x0, cfv[0])
                    nc.vector.tensor_mul(tb, x1, cfv[1])
                    nc.vector.tensor_sub(rot[:, :, 0, :], ta, tb)
                    tc2 = scrp.tile([128, HPC, 64], F16, tag="tc2")
                    td = scrp.tile([128, HPC, 64], F16, tag="td")
                    nc.vector.tensor_mul(tc2, x0, cfv[2])
                    nc.vector.tensor_mul(td, x1, cfv[3])
                    nc.vector.tensor_add(rot[:, :, 1, :], tc2, td)

            transposes(NT - 2)
            transposes(NT - 1)

        # ------------- phase 2+3: attention + output projection ----------
        with (
            tc.tile_pool(name="s_ps", bufs=2, space="PSUM") as sps,
            tc.tile_pool(name="o_ps", bufs=2, space="PSUM") as ops_,
            tc.tile_pool(name="bc_ps", bufs=2, space="PSUM") as bcps,
            tc.tile_pool(name="es", bufs=2) as esp,
            tc.tile_pool(name="tr1", bufs=2) as tr1p,
            tc.tile_pool(name="nrm", bufs=2) as nrmp,
            tc.tile_pool(name="ob", bufs=4) as obp,
        ):
            def norm_a(j, h, o_ps, acc512):
                # sums -> 1/sums (PE ones-matmul + DVE approx reciprocal)
                bc = bcps.tile([128, 512], F32, tag="bc")
                nc.tensor.matmul(
                    bc[0:1, :], lhsT=ones_col, rhs=acc512, start=True, stop=True
                )
                inv32 = nrmp.tile([1, 512], F32, tag="inv32")
                nc.vector.reciprocal_approx_fast(out=inv32, in_=bc[0:1, :])
                inv16 = nrmp.tile([1, 512], BF16, tag="inv16")
                nc.vector.tensor_copy(out=inv16, in_=inv32)
                return inv16

            def norm_b(j, h, o_ps, inv16):
                # broadcast 1/sums across partitions, O[j][:,h,:] = o_ps * inv
                bc = bcps.tile([128, 512], F32, tag="bc")
                nc.tensor.matmul(
                    bc, lhsT=ones_row, rhs=inv16, start=True, stop=True
                )
                invb = nrmp.tile([128, 512], BF16, tag="invb")
                nc.scalar.copy(out=invb, in_=bc)
                nc.vector.tensor_mul(O[j][:, h, :], o_ps, invb)

            def outproj_groups(j):
                fillers = []
                for it in range(4):
                    nsl = slice((4 * j + it) * 128, (4 * j + it + 1) * 128)
                    ob = obp.tile([128, 4, 512], F16, tag="ob", name="ob")
                    for dch in range(4):
                        def grp(j=j, it=it, dch=dch, ob=ob, nsl=nsl):
                            dsl = slice(dch * 512, (dch + 1) * 512)
                            op_ps = bcps.tile([128, 512], F32, tag="bc")
                            for h in range(HPC):
                                nc.tensor.matmul(
                                    op_ps,
                                    lhsT=O[j][:, h, it * 128 : (it + 1) * 128],
                                    rhs=wout_sb[:, h, dsl],
                                    start=(h == 0),
                                    stop=(h == HPC - 1),
                                )
                            if j == NJ - 1 or dch % 2 == 0:
                                nc.scalar.copy(out=ob[:, dch, :], in_=op_ps)
                            else:
                                nc.vector.tensor_copy(out=ob[:, dch, :], in_=op_ps)
                            if j == NJ - 1:
                                # tail: store per dch, alternating queues so
                                # the final transfers pipeline
                                dsl2 = slice(dch * 512, (dch + 1) * 512)
                                eng = nc.sync if dch % 2 else nc.gpsimd
                                eng.dma_start(
                                    out=outp[nsl, dsl2], in_=ob[:, dch, :]
                                )
                            elif dch == 3:
                                nc.gpsimd.dma_start(out=outp[nsl, :], in_=ob)
                        fillers.append(grp)
                return fillers

            fillers = []
            pend = None
            for j in range(NJ):
                qtile = QT[j // 2][:, :, (j % 2) * 512 : (j % 2) * 512 + 512]
                for h in range(HPC):
                    hsl = slice(h * HD, (h + 1) * HD)
                    o_ps = ops_.tile([128, 512], F32, tag="o")
                    es = {}

                    def pv(p):
                        for par in range(2):
                            m = 2 * p + par
                            nc.tensor.matmul(
                                o_ps,
                                lhsT=V[m // 4][:, m % 4, hsl],
                                rhs=es[p][:, par, :],
                                start=(m == 0),
                                stop=(m == NT - 1),
                            )

                    for p in range(MP):
                        s2 = sps.tile([128, 2, 512], F32, tag="s2")
                        for par in range(2):
                            m = 2 * p + par
                            nc.tensor.matmul(
                                s2[:, par, :],
                                lhsT=KT[m // 8][:, h, (m % 8) * 128 : (m % 8 + 1) * 128],
                                rhs=qtile[:, h, :],
                                start=True,
                                stop=True,
                            )
                        es_p = esp.tile([128, 2, 512], BF16, tag=f"es{p}")
                        if p == MP - 1:
                            # split the last pair's exp so PV can start on the
                            # first half while the second is still on ACT
                            nc.scalar.activation(
                                es_p[:, 0, :], s2[:, 0, :], AF.Exp,
                                scale=SCALE, bias=expoff_sb,
                            )
                            nc.scalar.activation(
                                es_p[:, 1, :], s2[:, 1, :], AF.Exp,
                                scale=SCALE, bias=expoff_sb,
                            )
                        else:
                            nc.scalar.activation(
                                es_p, s2, AF.Exp, scale=SCALE, bias=expoff_sb
                            )
                        es[p] = es_p
                        # normalize previous head in two stages so the PE never
                        # waits on the DVE reciprocal chain
                        if p == 1 and pend is not None:
                            pinv = norm_a(*pend)
                        if p == 5 and pend is not None:
                            norm_b(pend[0], pend[1], pend[2], pinv)
                            pend = None
                        if p >= 2:
                            pv(p - 2)
                        # previous chunk's output projection rides along once
                        # its last head has been normalized
                        if fillers and pend is None:
                            fillers.pop(0)()
                        # DVE running sum over exp tiles (short chain after
                        # the last exp, each add hides under a p-step)
                        if p == 1:
                            run = tr1p.tile([128, 2, 512], BF16, tag="run")
                            nc.vector.tensor_add(run, es[0], es[1])
                        elif p >= 2:
                            nc.vector.tensor_add(run, run, es[p])
                        if p == 7:
                            acc512 = tr1p.tile([128, 512], BF16, tag="acc")
                            nc.vector.tensor_add(acc512, run[:, 0, :], run[:, 1, :])
                    pv(MP - 2)
                    pv(MP - 1)
                    pend = (j, h, o_ps, acc512)
                for f in fillers:  # leftovers from chunk j-1
                    f()
                fillers = outproj_groups(j)
            pinv = norm_a(*pend)
            norm_b(pend[0], pend[1], pend[2], pinv)
            for f in fillers:  # last chunk's projection
                f()


_NC = None


def _get_nc():
    global _NC
    if _NC is None:
        nc = bacc.Bacc()
        with tile.TileContext(nc) as tc:
            _emit(tc)
        if not nc.is_finalized():
            nc.finalize()
        _NC = nc
    return _NC


_EO = np.concatenate([np.arange(0, HD, 2), np.arange(1, HD, 2)])
_PERM = np.concatenate([h * HD + _EO for h in range(HPC)])


def _prep_core(x, Wqkv, q_gamma, k_gamma, Wout, cos, sin, b, hg):
    hsl = slice(hg * CPC, (hg + 1) * CPC)
    # q/k rows deinterleaved to [even|odd] halves per head (rope 2x layout);
    # the shared permutation cancels in the q.k dot products
    Wq = Wqkv[0 * H * HD : 1 * H * HD][hsl][_PERM]
    Wk = Wqkv[1 * H * HD : 2 * H * HD][hsl][_PERM]
    Wv = Wqkv[2 * H * HD : 3 * H * HD][hsl]
    wqkvT = np.ascontiguousarray(np.concatenate([Wq, Wk, Wv], 0).T)
    woutT = np.ascontiguousarray(Wout[:, hsl].T)

    def c4(a):  # [N, 64] -> [N, 256] tiled over the 4 heads
        return np.tile(a, (1, HPC))

    qe, qo = q_gamma[0::2], q_gamma[1::2]
    ke, ko = k_gamma[0::2], k_gamma[1::2]
    cb, sb = cos[b], sin[b]  # [N, 64]
    coef = np.stack(
        [
            c4(cb * qe), c4(sb * qo), c4(sb * qe), c4(cb * qo),
            c4(cb * ke), c4(sb * ko), c4(sb * ke), c4(cb * ko),
        ],
        axis=1,
    ).astype(np.float16)  # [N, 8, 256]
    xt = x[b].T.astype(np.float16)  # [DIM, N]
    xt4 = np.ascontiguousarray(
        xt.reshape(DT, 128, 8, 256).transpose(1, 2, 0, 3)
    )  # [128, round, d, 256]
    wq4 = np.ascontiguousarray(
        wqkvT.astype(np.float16).reshape(DT, 128, 3, CPC).transpose(1, 2, 0, 3)
    )  # [128, c, d, CPC]
    cf4 = np.ascontiguousarray(
        coef.reshape(NT, 128, 8, 2 * HD).transpose(1, 0, 2, 3)
    )  # [128, tile, 8, 256]
    return {
        "xT": xt4,
        "wqkvT": wq4,
        "woutT": woutT.astype(np.float16),
        "coef": cf4,
    }


def prep_in_maps(x, Wqkv, q_gamma, k_gamma, Wout, freqs):
    x = np.asarray(x, np.float32)
    Wqkv = np.asarray(Wqkv, np.float32)
    Wout = np.asarray(Wout, np.float32)
    q_gamma = np.asarray(q_gamma, np.float32)
    k_gamma = np.asarray(k_gamma, np.float32)
    freqs = np.asarray(freqs, np.float32)
    cos = freqs[..., 0]
    sin = freqs[..., 1]
    return [
        _prep_core(x, Wqkv, q_gamma, k_gamma, Wout, cos, sin, c // GROUPS, c % GROUPS)
        for c in range(NCORES)
    ]


def gather(parts):
    out = np.empty((B, N, DIM), np.float32)
    for b in range(B):
        acc = parts[b * GROUPS].astype(np.float32)
        for g in range(1, GROUPS):
            acc = acc + parts[b * GROUPS + g].astype(np.float32)
        out[b] = acc
    return out


def kernel(x, Wqkv, q_gamma, k_gamma, Wout, freqs):
    from concourse.bass_utils import run_bass_kernel_spmd

    nc = _get_nc()
    in_maps = prep_in_maps(x, Wqkv, q_gamma, k_gamma, Wout, freqs)
    res = run_bass_kernel_spmd(nc, in_maps, list(range(NCORES)))
    parts = [res.results[c]["outp"] for c in range(NCORES)]
    return gather(parts)
